# revision 18
# baseline (speedup 1.0000x reference)
"""Trainium2 Bass kernel for causal self-attention with RoPE (Megatron-style
head-parallel over 8 NeuronCores).

Sharding: 16 heads / 8 cores = 2 heads per core. Wqkv split column-wise by
head; attention embarrassingly parallel over (batch, head); output projection
row-parallel with the partial contraction exchanged via AllToAll so core r
owns output rows [r*512, (r+1)*512) of the flattened [4096, 2048] output.

v4 (scheduling overhaul over v3):
- DMA queue separation: x chunks stream on the scalar HWDGE queue (2MB
  grains, fine-grained first chunk), weights + Wproj on the gpsimd SWDGE
  queue, small constants + rope copies + yt/out writes on the sync queue.
  All three drain in parallel so phase 1 starts at ~4us and never starves.
- the AllToAll gathers (a2a_out -> SBUF) live at the TAIL of the scalar
  queue behind a no_sync_barrier: they park nothing (scalar has no compute
  after attention), fixing the 34us sync-queue park of v3 where the yts0
  gather blocked head-1's normalize pipeline and stalled the PE 20us.
- head-0 A2A kept whole (fewer collective floors); head-1 split in two
  token-halves so the projection stage-B gets its first data earlier.
- projection runs an 8-deep accumulator pipeline (all 8 PSUM banks) with an
  8-pair stage-A prefix to ride out the head-1 collective latency.

All matmuls bf16 with fp32 PSUM accumulation. Softmax skips max-subtraction
(scores are O(+-10) here) and computes the denominator with a ones-row
matmul, software-pipelined one chunk behind the PE.
"""

import sys

if "/opt/trn_rl_repo" not in sys.path:
    sys.path.insert(0, "/opt/trn_rl_repo")

import ml_dtypes
import numpy as np

import concourse.bacc as bacc
import concourse.bass as bass
import concourse.mybir as mybir
import concourse.tile as tile
from concourse.bass_utils import run_bass_kernel_spmd

B, T, C, H, D = 4, 1024, 2048, 16, 128
TQ = B * T           # 4096 flattened tokens
NCORES = 8
HPC = H // NCORES    # heads per core = 2
FQK = 4 * D          # 512 qkT feature rows per core (qa, qb, ka, kb)
FV = HPC * D         # 256 v feature cols per core
ROWS = TQ // NCORES  # 512 output rows per core
NCT = C // 128       # 16 contraction tiles
SCALE = 1.0 / float(np.sqrt(D))

F32 = mybir.dt.float32
BF16 = mybir.dt.bfloat16

_CACHE = {}


def _build_program():
    nc = bacc.Bacc(
        "TRN2",
        target_bir_lowering=False,
        debug=False,
        enable_asserts=False,
        num_devices=NCORES,
    )

    # ---- I/O (all big tensors pre-tiled [partition, ...] bf16 on host) ----
    xt = nc.dram_tensor("xt", [128, 8, NCT, 512], BF16, kind="ExternalInput")
    wqk = nc.dram_tensor("wqk", [128, NCT, FQK], BF16, kind="ExternalInput")
    wv = nc.dram_tensor("wv", [128, NCT, FV], BF16, kind="ExternalInput")
    bqk = nc.dram_tensor("bqk", [128, 4], F32, kind="ExternalInput")
    bv = nc.dram_tensor("bv", [128, FV], BF16, kind="ExternalInput")
    wp = nc.dram_tensor("wp", [128, 4, NCT, 512], BF16, kind="ExternalInput")
    bproj = nc.dram_tensor("bproj", [128, C], BF16, kind="ExternalInput")
    cosd = nc.dram_tensor("cosd", [128, T], BF16, kind="ExternalInput")
    sind = nc.dram_tensor("sind", [128, T], BF16, kind="ExternalInput")
    out = nc.dram_tensor("out", [ROWS, C], BF16, kind="ExternalOutput")

    NT = TQ // 512  # 8 token chunks of 512
    Exp = mybir.ActivationFunctionType.Exp
    add = mybir.AluOpType.add
    mult = mybir.AluOpType.mult

    with tile.TileContext(nc) as tc:
        with (
            tc.tile_pool(name="const", bufs=1) as cpool,
            tc.tile_pool(name="resident", bufs=1) as rpool,
            tc.tile_pool(name="work", bufs=2) as wpool,
            tc.tile_pool(name="att", bufs=2) as apool,
            tc.tile_pool(name="psA", bufs=2, space="PSUM") as psA,
            tc.tile_pool(name="psB", bufs=2, space="PSUM") as psB,
            tc.tile_pool(name="dram", bufs=1, space="DRAM") as dpool,
        ):
            # ---- small constants first on the sync queue (first stt needs
            # bqk + sin within ~3us) --------------------------------------
            bqk_sb = cpool.tile([128, 4], F32)
            nc.sync.dma_start(out=bqk_sb[:], in_=bqk[:])
            sin_sb = cpool.tile([128, T], BF16)
            nc.sync.dma_start(out=sin_sb[:], in_=sind[:])
            cos_sb = cpool.tile([128, T], BF16)
            nc.sync.dma_start(out=cos_sb[:], in_=cosd[:])
            bv_sb = cpool.tile([128, FV], BF16)
            nc.sync.dma_start(out=bv_sb[:], in_=bv[:])
            bproj_sb = cpool.tile([128, C], BF16)
            nc.sync.dma_start(out=bproj_sb[:], in_=bproj[:])

            # ---- weights on the gpsimd SWDGE queue: fine first grain so
            # the first accumulation group starts after ~300KB -------------
            wqk_sb = cpool.tile([128, NCT, FQK], BF16, tag="wqk")
            nc.gpsimd.dma_start(out=wqk_sb[:, 0:2, :], in_=wqk[:, 0:2, :])
            nc.gpsimd.dma_start(out=wqk_sb[:, 2:8, :], in_=wqk[:, 2:8, :])
            nc.gpsimd.dma_start(out=wqk_sb[:, 8:16, :], in_=wqk[:, 8:16, :])
            wv_sb = cpool.tile([128, NCT, FV], BF16, tag="wv")
            nc.gpsimd.dma_start(out=wv_sb[:], in_=wv[:])

            # attention constants early (gpsimd is otherwise idle now)
            ones_sb = cpool.tile([128, 1], BF16)
            nc.gpsimd.memset(ones_sb[:], 1.0)
            mask_sb = cpool.tile([128, 4, 512], BF16)
            nc.gpsimd.memset(mask_sb[:], 1.0)
            for m in range(4):
                nc.gpsimd.affine_select(
                    out=mask_sb[:, m, :],
                    in_=mask_sb[:, m, :],
                    compare_op=mybir.AluOpType.is_ge,
                    fill=0.0,
                    base=-128 * m,
                    pattern=[[1, 512]],
                    channel_multiplier=-1,
                )

            # Wproj e-chunks (loads emitted mid-phase-1 so they don't steal
            # HBM bandwidth from the x/wqk stream the PE needs first)
            wp_tiles = [
                cpool.tile([128, NCT, 512], BF16, tag=f"wp{ec}", name=f"wp{ec}")
                for ec in range(4)
            ]

            # ---- x chunks stream on the scalar HWDGE queue (big grains;
            # chunk 0 split fine so the PE starts at ~4us) -----------------
            xt_tiles = {}
            for ch in range(NT):
                xt_tiles[ch] = wpool.tile(
                    [128, NCT, 512], BF16, tag="xT_ch", bufs=2,
                    name=f"xT_ch{ch}",
                )
            # Wproj loads are interleaved between x chunks on this queue:
            # the xT_ch bufs=2 slot-release rhythm paces them so they never
            # steal HBM bandwidth from the x stream the PE is waiting on.
            nc.scalar.dma_start(out=xt_tiles[0][:, 0:2, :], in_=xt[:, 0, 0:2, :])
            nc.scalar.dma_start(out=xt_tiles[0][:, 2:8, :], in_=xt[:, 0, 2:8, :])
            nc.scalar.dma_start(out=xt_tiles[0][:, 8:16, :], in_=xt[:, 0, 8:16, :])
            nc.scalar.dma_start(out=xt_tiles[1][:, 0:8, :], in_=xt[:, 1, 0:8, :])
            nc.scalar.dma_start(out=xt_tiles[1][:, 8:16, :], in_=xt[:, 1, 8:16, :])
            for ch in range(2, NT):
                nc.scalar.dma_start(out=xt_tiles[ch][:], in_=xt[:, ch, :, :])

            # ---- phase 1: QKV projection + RoPE --------------------------
            qkT_sb = rpool.tile([128, 4, TQ], BF16, tag="qkT")
            v_sb = rpool.tile([128, TQ // 128, FV], BF16)

            for ch in range(NT):
                t0 = ch * 512
                tc0 = (ch % 2) * 512  # position within the batch for RoPE
                xT_ch = xt_tiles[ch]
                # contraction-outer: each x piece feeds 4 live PSUM groups
                psq1 = psA.tile([128, 2, 512], F32, tag="mm512", name="psq1")
                psq2 = psA.tile([128, 2, 512], F32, tag="mm512", name="psq2")
                qps = [psq1[:, 0, :], psq1[:, 1, :], psq2[:, 0, :], psq2[:, 1, :]]
                for ct in range(NCT):
                    for mi in range(4):
                        nc.tensor.matmul(
                            qps[mi],
                            lhsT=wqk_sb[:, ct, mi * 128 : (mi + 1) * 128],
                            rhs=xT_ch[:, ct, :],
                            start=(ct == 0),
                            stop=(ct == NCT - 1),
                        )
                # evict + bias + RoPE; rotate-half via two half-partition
                # SBUF->SBUF DMA copies (sin table carries the sign)
                m1s, m2ss = [None] * 4, [None] * 4
                for mi in range(4):
                    m2 = wpool.tile([128, 512], BF16, tag="rope_m2", bufs=3)
                    nc.vector.scalar_tensor_tensor(
                        out=m2[:], in0=qps[mi], scalar=bqk_sb[:, mi : mi + 1],
                        in1=sin_sb[:, tc0 : tc0 + 512], op0=add, op1=mult,
                    )
                    m2s = wpool.tile([128, 512], BF16, tag="rope_m2s", bufs=3)
                    nc.sync.dma_start(out=m2s[0:64, :], in_=m2[64:128, :])
                    nc.sync.dma_start(out=m2s[64:128, :], in_=m2[0:64, :])
                    m1 = wpool.tile([128, 512], BF16, tag="rope_m1", bufs=3)
                    nc.vector.scalar_tensor_tensor(
                        out=m1[:], in0=qps[mi], scalar=bqk_sb[:, mi : mi + 1],
                        in1=cos_sb[:, tc0 : tc0 + 512], op0=add, op1=mult,
                    )
                    m1s[mi], m2ss[mi] = m1, m2s
                    if mi >= 1:
                        nc.vector.tensor_add(
                            qkT_sb[:, mi - 1, t0 : t0 + 512],
                            m1s[mi - 1][:], m2ss[mi - 1][:],
                        )
                nc.vector.tensor_add(
                    qkT_sb[:, 3, t0 : t0 + 512], m1s[3][:], m2ss[3][:]
                )
                for tt in range(4):
                    psv = psB.tile([128, 512], F32, tag="acc")
                    for ct in range(NCT):
                        nc.tensor.matmul(
                            psv[:, 0:FV],
                            lhsT=xT_ch[:, ct, tt * 128 : (tt + 1) * 128],
                            rhs=wv_sb[:, ct, :],
                            start=(ct == 0),
                            stop=(ct == NCT - 1),
                        )
                    nc.vector.tensor_add(
                        v_sb[:, ch * 4 + tt, :], psv[:, 0:FV], bv_sb[:]
                    )
                if ch == 2:
                    for ec in range(4):
                        nc.gpsimd.dma_start(
                            out=wp_tiles[ec][:], in_=wp[:, ec, :, :]
                        )

            # ---- phase 2: attention, head-outer, normalize pipelined ----
            a2a_in = [
                [
                    dpool.tile([NCORES, 128, 256], BF16, name=f"a2a_in{hl}{h}")
                    for h in range(2)
                ]
                for hl in range(2)
            ]
            # gathered features: yts0[p, g, t] = head (2g) feature p of my
            # token t; yts1 likewise for heads (2g+1)
            yts0 = cpool.tile([128, NCORES, 512], BF16, tag="wqk", name="yts0")

            pending = [None]  # deferred tail of the previous chunk's softmax
            a2a_out = [[None, None], [None, None]]

            def norm_finish():
                ot_ps, denb, p, hl = pending[0]
                pending[0] = None
                yt = apool.tile([128, 512], BF16, tag="yt")
                nc.vector.tensor_mul(yt[:], ot_ps, denb)
                nc.sync.dma_start(out=a2a_in[hl][0][p, :, :], in_=yt[:, 0:256])
                nc.sync.dma_start(
                    out=a2a_in[hl][1][p, :, :], in_=yt[:, 256:512]
                )

            for hl in range(HPC):
                qh = qkT_sb[:, hl, :]
                kh = qkT_sb[:, 2 + hl, :]
                for b in range(B):
                    for tqc in range(2):
                        tq0 = b * T + tqc * 512
                        nj = 4 * (tqc + 1)
                        ot_ps = psB.tile([128, 512], F32, tag="acc")
                        den_ps = psB.tile([1, 512], F32, tag="aux")

                        def ot_den(jp, ptp):
                            # attn@v + denominator matmuls for one j-pair
                            for jj in range(2):
                                j = 2 * jp + jj
                                m = j - (nj - 4)
                                w0 = 128 * m if m > 0 else 0
                                pt = ptp[:, jj, w0:512]
                                vt = v_sb[
                                    :, b * 8 + j, hl * 128 : (hl + 1) * 128
                                ]
                                nc.tensor.matmul(
                                    ot_ps[:, w0:512], lhsT=vt, rhs=pt,
                                    start=(j == 0), stop=(j == nj - 1),
                                )
                                nc.tensor.matmul(
                                    den_ps[:, w0:512], lhsT=ones_sb[:],
                                    rhs=pt,
                                    start=(j == 0), stop=(j == nj - 1),
                                )

                        for jp in range(nj // 2):
                            st_ps = psA.tile([128, 2, 512], F32, tag="mm512")
                            ptp = apool.tile(
                                [128, 2, 512], BF16, tag="pt", bufs=3
                            )
                            # per-jj scores -> exp: the exp of jj0 runs
                            # while the PE computes jj1's scores, so the
                            # attn@v matmuls wait on at most half an exp
                            for jj in range(2):
                                j = 2 * jp + jj
                                m = j - (nj - 4)
                                w0 = 128 * m if m > 0 else 0
                                s0 = b * T + j * 128
                                nc.tensor.matmul(
                                    st_ps[:, jj, w0:512],
                                    lhsT=kh[:, s0 : s0 + 128],
                                    rhs=qh[:, tq0 + w0 : tq0 + 512],
                                    start=True,
                                    stop=True,
                                )
                                nc.scalar.activation(
                                    ptp[:, jj, w0:512],
                                    st_ps[:, jj, w0:512],
                                    Exp,
                                    scale=SCALE,
                                )
                                if m >= 0:
                                    nc.vector.tensor_mul(
                                        ptp[:, jj, w0:512],
                                        ptp[:, jj, w0:512],
                                        mask_sb[:, m, w0:512],
                                    )
                            ot_den(jp, ptp)
                        if pending[0] is not None:
                            norm_finish()
                        # this chunk's normalize: reciprocal straight from
                        # PSUM on the DVE (bf16 row), partition-broadcast
                        # on GpSimd; the mul runs at the next chunk's end
                        recip_row = apool.tile(
                            [1, 512], F32, tag="recip_row", bufs=2
                        )
                        nc.vector.reciprocal_approx_fast(
                            recip_row[:], den_ps[:]
                        )
                        denb = apool.tile(
                            [128, 512], F32, tag="denb", name="denb"
                        )
                        nc.gpsimd.partition_broadcast(denb[:], recip_row[:])
                        pending[0] = (ot_ps, denb[:], b * 2 + tqc, hl)
                # exchange this head's features (two token-half AllToAlls)
                # while the rest of the kernel keeps the PE busy
                norm_finish()
                for h in range(2):
                    a_out = dpool.tile(
                        [NCORES, 128, 256], BF16, name=f"a2a_out{hl}{h}"
                    )
                    a2a_out[hl][h] = a_out
                    nc.gpsimd.collective_compute(
                        "AllToAll",
                        mybir.AluOpType.bypass,
                        replica_groups=[list(range(NCORES))],
                        ins=[a2a_in[hl][h][:].opt()],
                        outs=[a_out[:].opt()],
                    )

            # ---- gathers: tail of the scalar queue, fenced so the
            # scheduler cannot hoist them over attention work ------------
            tc.no_sync_barrier()
            yts1 = rpool.tile([128, NCORES, 512], BF16, tag="qkT", name="yts1")
            for hl, dst in ((0, yts0), (1, yts1)):
                for h, tsl in ((0, slice(0, 256)), (1, slice(256, 512))):
                    nc.scalar.dma_start(
                        out=dst[:, :, tsl],
                        in_=a2a_out[hl][h].rearrange("g p t -> p g t"),
                    )

            # ---- phase 3: projection, 8-deep accumulator pipeline -------
            # stage A contracts the 8 head-0 feature tiles, stage B the 8
            # head-1 tiles as the half-collectives land.  Groups are run
            # two at a time with their matmuls interleaved so consecutive
            # MMs target different PSUM banks (a same-bank accumulation
            # chain pays the ~128-cycle systolic drain per matmul).
            pairs = [(ec, tt) for tt in (0, 1) for ec in range(4)] + [
                (ec, tt) for tt in (2, 3) for ec in range(4)
            ]
            accA1 = psA.tile([128, 2, 512], F32, tag="mm512", name="accA1")
            accA2 = psA.tile([128, 2, 512], F32, tag="mm512", name="accA2")
            accB1 = psB.tile([128, 512], F32, tag="acc", name="accB1")
            accB2 = psB.tile([128, 512], F32, tag="acc", name="accB2")
            accC1 = psB.tile([128, 512], F32, tag="aux", name="accC1")
            accC2 = psB.tile([128, 512], F32, tag="aux", name="accC2")
            slots = [
                accA1[:, 0, :], accA1[:, 1, :],
                accA2[:, 0, :], accA2[:, 1, :],
                accB1[:, :], accB2[:, :],
                accC1[:, :], accC2[:, :],
            ]

            def mm_a(i, g):
                ec, tt = pairs[i]
                nc.tensor.matmul(
                    slots[i % 8],
                    lhsT=yts0[:, g, tt * 128 : (tt + 1) * 128],
                    rhs=wp_tiles[ec][:, 2 * g, :],
                    start=(g == 0),
                    stop=False,
                )

            def mm_b(i, g):
                ec, tt = pairs[i]
                nc.tensor.matmul(
                    slots[i % 8],
                    lhsT=yts1[:, g, tt * 128 : (tt + 1) * 128],
                    rhs=wp_tiles[ec][:, 2 * g + 1, :],
                    start=False,
                    stop=(g == NCORES - 1),
                )

            def evict(i):
                ec, tt = pairs[i]
                osb = wpool.tile([128, 512], BF16, tag="osb")
                nc.vector.tensor_add(
                    osb[:], slots[i % 8],
                    bproj_sb[:, ec * 512 : (ec + 1) * 512],
                )
                nc.sync.dma_start(
                    out=out[tt * 128 : (tt + 1) * 128, ec * 512 : (ec + 1) * 512],
                    in_=osb[:],
                )

            brackets = (
                [[("a", 2 * k), ("a", 2 * k + 1)] for k in range(4)]
                + [
                    grp
                    for k in range(4)
                    for grp in (
                        [("b", 2 * k), ("b", 2 * k + 1)],
                        [("a", 8 + 2 * k), ("a", 8 + 2 * k + 1)],
                    )
                ]
                + [[("b", 8 + 2 * k), ("b", 8 + 2 * k + 1)] for k in range(4)]
            )
            for grp in brackets:
                for g in range(NCORES):
                    for kind, i in grp:
                        (mm_a if kind == "a" else mm_b)(i, g)
                for kind, i in grp:
                    if kind == "b":
                        evict(i)

    nc.compile()
    return nc


def _rope_tables():
    inv = 1.0 / (10000.0 ** (np.arange(0, D, 2, dtype=np.float64) / D))
    t = np.arange(T, dtype=np.float64)
    fr = np.outer(t, inv)  # [T, 64]
    cosT = np.tile(np.cos(fr).T, (2, 1))
    # rotate-half runs as a pure half-partition swap; the sign of the sin
    # term is folded into the table (source rows >= 64 land negated)
    sinT = np.tile(np.sin(fr).T, (2, 1))
    sinT[64:128] *= -1.0
    bf16 = ml_dtypes.bfloat16
    return (
        np.ascontiguousarray(cosT.astype(bf16)),
        np.ascontiguousarray(sinT.astype(bf16)),
    )


def _prep_inputs(x, Wqkv, bqkv, Wproj, bproj):
    bf16 = ml_dtypes.bfloat16
    x2 = np.asarray(x, np.float32).reshape(TQ, C)
    Wqkv = np.asarray(Wqkv, np.float32)
    bqkv = np.asarray(bqkv, np.float32)
    Wproj = np.asarray(Wproj, np.float32)
    bproj = np.asarray(bproj, np.float32)

    # x^T pre-tiled: [p, ch, ct, t] = x[ch*512+t, ct*128+p]
    xt = np.ascontiguousarray(
        x2.T.reshape(NCT, 128, 8, 512).transpose(1, 2, 0, 3).astype(bf16)
    )
    # Wproj pre-tiled: [p, ec, ft, e] = Wproj[ft*128+p, ec*512+e]
    wp_t = np.ascontiguousarray(
        Wproj.reshape(NCT, 128, 4, 512).transpose(1, 2, 0, 3).astype(bf16)
    )
    cosT, sinT = _rope_tables()
    bproj_b = np.ascontiguousarray(
        np.broadcast_to(bproj[None, :], (128, C)).astype(bf16)
    )

    Wq = Wqkv[:, 0 * C : 1 * C].reshape(C, H, D)
    Wk = Wqkv[:, 1 * C : 2 * C].reshape(C, H, D)
    Wv = Wqkv[:, 2 * C : 3 * C].reshape(C, H, D)
    bq = bqkv[0 * C : 1 * C].reshape(H, D)
    bk = bqkv[1 * C : 2 * C].reshape(H, D)
    bv_ = bqkv[2 * C : 3 * C].reshape(H, D)

    in_maps = []
    for r in range(NCORES):
        ha, hb = 2 * r, 2 * r + 1
        wqk_s = np.concatenate(
            [Wq[:, ha], Wq[:, hb], Wk[:, ha], Wk[:, hb]], axis=1
        )
        wqk_t = np.ascontiguousarray(
            wqk_s.reshape(NCT, 128, FQK).transpose(1, 0, 2).astype(bf16)
        )
        bqk_s = np.ascontiguousarray(
            np.stack([bq[ha], bq[hb], bk[ha], bk[hb]], axis=1)
        )  # [128, 4]
        wv_s = np.concatenate([Wv[:, ha], Wv[:, hb]], axis=1)
        wv_t = np.ascontiguousarray(
            wv_s.reshape(NCT, 128, FV).transpose(1, 0, 2).astype(bf16)
        )
        bv_s = np.ascontiguousarray(
            np.broadcast_to(
                np.concatenate([bv_[ha], bv_[hb]])[None, :], (128, FV)
            ).astype(bf16)
        )
        in_maps.append(
            {
                "xt": xt,
                "wqk": wqk_t,
                "wv": wv_t,
                "bqk": bqk_s,
                "bv": bv_s,
                "wp": wp_t,
                "bproj": bproj_b,
                "cosd": cosT,
                "sind": sinT,
            }
        )
    return in_maps


def kernel(x, Wqkv, bqkv, Wproj, bproj, _trace=False, _trace_kwargs=None):
    if "nc" not in _CACHE:
        _CACHE["nc"] = _build_program()
    nc = _CACHE["nc"]
    in_maps = _prep_inputs(x, Wqkv, bqkv, Wproj, bproj)
    kwargs = {}
    if _trace:
        kwargs.update(trace=True, **(_trace_kwargs or {}))
    res = run_bass_kernel_spmd(nc, in_maps, core_ids=list(range(NCORES)), **kwargs)
    _CACHE["last_results"] = res
    out = np.concatenate(
        [res.results[r]["out"].astype(np.float32) for r in range(NCORES)],
        axis=0,
    )
    return np.ascontiguousarray(out.reshape(B, T, C))
